# revision 5
# baseline (speedup 1.0000x reference)
"""Trainium2 Bass kernel for nn_BDHTinyModel (4-layer weight-shared tiny transformer).

Sharding: 8 NeuronCores = 4 batch groups x 2 tensor-parallel halves.
Core c handles batch b=c//2 and heads [4*(c%2), 4*(c%2)+4). After the
y@encoder projection each pair all-reduces the partial v-update; each core
computes logits for half the vocab.

Key layout trick: activations are kept feature-on-partition ([n, t] / [d, t])
for matmuls; rope is made lane-local by de-interleaving even/odd channels
host-side (weights' columns, encoder's rows are permuted to match, which is
exact because the n-contraction is order-invariant).
"""

import math

import numpy as np
import ml_dtypes

import concourse.bass as bass
import concourse.mybir as mybir
import concourse.tile as tile
from concourse import bacc
from concourse.bass_utils import run_bass_kernel_spmd
from concourse.masks import make_identity

# model dims (hardcoded per the problem spec)
B, T, D, NH, N, VOCAB, NL = 4, 1024, 512, 8, 1024, 32000, 4
EPS = 1e-5
P = 128
NHL = NH // 2          # heads per core
VLOC = VOCAB // 2      # vocab half per core
N_CORES = 8
GROUPS = [[0, 1], [2, 3], [4, 5], [6, 7]]

f32 = mybir.dt.float32
bf16 = mybir.dt.bfloat16
i32 = mybir.dt.int32
Alu = mybir.AluOpType
Act = mybir.ActivationFunctionType
AX = mybir.AxisListType


def build_nc():
    nc = bacc.Bacc(num_devices=N_CORES)

    EMB = nc.declare_dram_parameter("emb", [VOCAB, D], f32, isOutput=False)
    IDX = nc.declare_dram_parameter("idx", [P, T // P], i32, isOutput=False)
    WX = nc.declare_dram_parameter("wx", [P, NHL, 8, 4, P], bf16, isOutput=False)
    WY = nc.declare_dram_parameter("wy", [P, NHL, 8, 4, P], bf16, isOutput=False)
    ENC = nc.declare_dram_parameter("enc", [P, NHL, 8, D], bf16, isOutput=False)
    LMH = nc.declare_dram_parameter("lmh", [P, 4, VLOC], bf16, isOutput=False)
    COS = nc.declare_dram_parameter("cos", [P, 4, T], bf16, isOutput=False)
    SIN = nc.declare_dram_parameter("sin", [P, 4, T], bf16, isOutput=False)
    MASK = nc.declare_dram_parameter("mask", [P, 4, 512], f32, isOutput=False)
    OUT = nc.declare_dram_parameter("out", [T, VLOC], f32, isOutput=True)

    with tile.TileContext(nc) as tc:
        with (
            tc.tile_pool(name="wpool", bufs=1) as wpool,
            tc.tile_pool(name="vpool", bufs=1) as vpool,
            tc.tile_pool(name="spool", bufs=4) as spool,
            tc.tile_pool(name="mmps", bufs=5, space="PSUM") as mmps,
            tc.tile_pool(name="stps", bufs=2, space="PSUM") as stps,
            tc.tile_pool(name="drp", bufs=2, space="DRAM") as drp,
        ):
            # ---- resident constants/weights ----
            cos_sb = wpool.tile([P, 4, T], bf16)
            nc.sync.dma_start(cos_sb[:], COS[:])
            sin_sb = wpool.tile([P, 4, T], bf16)
            nc.sync.dma_start(sin_sb[:], SIN[:])
            mask_sb = wpool.tile([P, 4, 512], f32)
            nc.sync.dma_start(mask_sb[:], MASK[:])
            idx_sb = wpool.tile([P, T // P], i32)
            nc.sync.dma_start(idx_sb[:], IDX[:])
            ident = wpool.tile([P, P], bf16)
            make_identity(nc, ident[:])
            ones = wpool.tile([P, 1], bf16)
            nc.vector.memset(ones[:], 1.0)
            epsb = wpool.tile([P, 1], f32)
            nc.vector.memset(epsb[:], EPS)

            # ---- v state ----
            v32 = vpool.tile([P, 8, D], f32)    # v in [t, d] layout, fp32
            vbf = vpool.tile([P, 8, D], bf16)   # bf16 copy (matmul rhs/lhsT)
            vT = vpool.tile([P, 4, T], bf16)    # v^T in [d, t] layout

            def transpose_v():
                # vbf [t,d] -> vT [d,t], 32 PE transposes of 128x128
                for tt in range(8):
                    for ds in range(4):
                        ps = mmps.tile([P, P], bf16, tag="mm", name="tp_ps")
                        nc.tensor.transpose(
                            ps[:], vbf[:, tt, ds * P:(ds + 1) * P], ident[:]
                        )
                        nc.scalar.copy(vT[:, ds, tt * P:(tt + 1) * P], ps[:])

            # ---- embedding gather + LayerNorm -> v0 ----
            with tc.tile_pool(name="gpool", bufs=2) as gpool:
                for tt in range(8):
                    g = gpool.tile([P, D], f32, tag="gather")
                    nc.gpsimd.indirect_dma_start(
                        out=g[:],
                        out_offset=None,
                        in_=EMB[:],
                        in_offset=bass.IndirectOffsetOnAxis(
                            ap=idx_sb[:, tt:tt + 1], axis=0
                        ),
                    )
                    s = spool.tile([P, 1], f32, tag="s")
                    nc.vector.tensor_reduce(s[:], g[:], axis=AX.X, op=Alu.add)
                    q = spool.tile([P, 1], f32, tag="q")
                    dummy = gpool.tile([P, D], bf16, tag="sqd")
                    nc.scalar.activation(dummy[:], g[:], Act.Square, accum_out=q[:])
                    mu = spool.tile([P, 1], f32, tag="mu")
                    nc.scalar.mul(mu[:], s[:], 1.0 / D)
                    mu2 = spool.tile([P, 1], f32, tag="mu2")
                    nc.vector.tensor_tensor(mu2[:], mu[:], mu[:], op=Alu.mult)
                    var = spool.tile([P, 1], f32, tag="var")
                    nc.vector.scalar_tensor_tensor(
                        var[:], q[:], 1.0 / D, mu2[:], Alu.mult, Alu.subtract
                    )
                    std = spool.tile([P, 1], f32, tag="std")
                    nc.scalar.activation(std[:], var[:], Act.Sqrt, bias=epsb[:])
                    rsq = spool.tile([P, 1], f32, tag="rsq")
                    nc.vector.reciprocal(rsq[:], std[:])
                    nc.vector.tensor_scalar(
                        v32[:, tt, :], g[:], mu[:], rsq[:], Alu.subtract, Alu.mult
                    )
                    nc.scalar.copy(vbf[:, tt, :], v32[:, tt, :])
            transpose_v()

            # ---- layers ----
            with (
                tc.tile_pool(name="slab", bufs=3) as slab,
                tc.tile_pool(name="atp", bufs=1) as atp,
                tc.tile_pool(name="scrp", bufs=1) as scrp,
                tc.tile_pool(name="sqp", bufs=4) as sqp,
                tc.tile_pool(name="encp", bufs=10) as encp,
                tc.tile_pool(name="wxp", bufs=2) as wxp,
                tc.tile_pool(name="wyp", bufs=2) as wyp,
            ):
                for layer in range(NL):
                    vd = vpool.tile([P, 8, D], f32, tag="vd", name=f"vd_{layer}")
                    for h in range(NHL):
                        # -- x^T = relu(Wx^T @ v^T), rope-permuted [n, t] layout --
                        XT = slab.tile([P, 8, T], bf16, tag="slab", name=f"xt_{layer}_{h}")
                        wxh = wxp.tile([P, 8, 4, P], bf16, tag="wx", name=f"wx_{layer}_{h}")
                        nc.sync.dma_start(wxh[:], WX[:, h])
                        for nt in range(8):
                            for c in range(2):
                                ps = mmps.tile([P, 512], f32, tag="mm", name="x_ps")
                                for ds in range(4):
                                    nc.tensor.matmul(
                                        ps[:],
                                        wxh[:, nt, ds, :],
                                        vT[:, ds, c * 512:(c + 1) * 512],
                                        start=(ds == 0),
                                        stop=(ds == 3),
                                    )
                                nc.scalar.activation(
                                    XT[:, nt, c * 512:(c + 1) * 512], ps[:], Act.Relu
                                )
                        # -- rope: QA = XE*cos - XO*sin ; QB = XE*sin + XO*cos --
                        QR = slab.tile([P, 8, T], bf16, tag="slab", name=f"qr_{layer}_{h}")
                        tmp = scrp.tile([P, 4, T], bf16, tag="scr", name="rope_tmp")
                        nc.vector.tensor_tensor(QR[:, 0:4, :], XT[:, 0:4, :], cos_sb[:], op=Alu.mult)
                        nc.vector.tensor_tensor(tmp[:], XT[:, 4:8, :], sin_sb[:], op=Alu.mult)
                        nc.vector.tensor_tensor(QR[:, 0:4, :], QR[:, 0:4, :], tmp[:], op=Alu.subtract)
                        nc.vector.tensor_tensor(QR[:, 4:8, :], XT[:, 0:4, :], sin_sb[:], op=Alu.mult)
                        tmp2 = scrp.tile([P, 4, T], bf16, tag="scr", name="rope_tmp2")
                        nc.vector.tensor_tensor(tmp2[:], XT[:, 4:8, :], cos_sb[:], op=Alu.mult)
                        nc.vector.tensor_tensor(QR[:, 4:8, :], QR[:, 4:8, :], tmp2[:], op=Alu.add)

                        # -- scores^T (strictly causal s<t), symmetric trick --
                        ST = slab.tile([P, 8, T], bf16, tag="slab", name=f"st_{layer}_{h}")
                        for k in range(8):
                            for c in range(2):
                                if k * 128 >= (c + 1) * 512:
                                    continue
                                ps = mmps.tile([P, 512], f32, tag="mm", name="s_ps")
                                for ns in range(8):
                                    nc.tensor.matmul(
                                        ps[:],
                                        QR[:, ns, k * 128:(k + 1) * 128],
                                        QR[:, ns, c * 512:(c + 1) * 512],
                                        start=(ns == 0),
                                        stop=(ns == 7),
                                    )
                                if (k + 1) * 128 <= c * 512:
                                    nc.scalar.copy(ST[:, k, c * 512:(c + 1) * 512], ps[:])
                                else:
                                    nc.vector.tensor_tensor(
                                        ST[:, k, c * 512:(c + 1) * 512],
                                        ps[:],
                                        mask_sb[:, k - 4 * c, :],
                                        op=Alu.mult,
                                    )

                        # -- a^T = v^T @ S_masked  ([d, t] layout) + rms stats --
                        AT = atp.tile([P, 4, T], bf16, tag="at", name=f"at_{layer}_{h}")
                        rsqh = spool.tile([P, 8], f32, tag="rsqh", name=f"rsqh_{layer}_{h}")
                        std8 = spool.tile([P, 8], f32, tag="std8", name=f"std8_{layer}_{h}")
                        for c in range(2):
                            sqs = []
                            for dt in range(4):
                                ps = mmps.tile([P, 512], f32, tag="mm", name="a_ps")
                                nss = range(4) if c == 0 else range(8)
                                last = len(nss) - 1
                                for i, ns in enumerate(nss):
                                    nc.tensor.matmul(
                                        ps[:],
                                        vbf[:, ns, dt * 128:(dt + 1) * 128],
                                        ST[:, ns, c * 512:(c + 1) * 512],
                                        start=(i == 0),
                                        stop=(i == last),
                                    )
                                nc.scalar.copy(AT[:, dt, c * 512:(c + 1) * 512], ps[:])
                                sq = sqp.tile([P, 512], bf16, tag="sq", name="sq")
                                nc.scalar.square(sq[:], ps[:])
                                sqs.append(sq)
                            # column sums of a^2 via ones-matmul -> [t,1] stats
                            for tq in range(4):
                                tt = c * 4 + tq
                                sps = stps.tile([P, 1], f32, tag="stat", name="a_stat")
                                for dt in range(4):
                                    nc.tensor.matmul(
                                        sps[:],
                                        sqs[dt][:, tq * 128:(tq + 1) * 128],
                                        ones[:],
                                        start=(dt == 0),
                                        stop=(dt == 3),
                                    )
                                nc.scalar.activation(
                                    std8[:, tt:tt + 1], sps[:], Act.Sqrt,
                                    bias=epsb[:], scale=1.0 / D,
                                )
                                nc.vector.reciprocal(rsqh[:, tt:tt + 1], std8[:, tt:tt + 1])

                        # -- y = relu(ln(a) @ Wy) * x ; rsq factored out --
                        YT = slab.tile([P, 8, T], bf16, tag="slab", name=f"yt_{layer}_{h}")
                        wyh = wyp.tile([P, 8, 4, P], bf16, tag="wy", name=f"wy_{layer}_{h}")
                        nc.sync.dma_start(wyh[:], WY[:, h])
                        for nt in range(8):
                            for c in range(2):
                                ps = mmps.tile([P, 512], f32, tag="mm", name="z_ps")
                                for ds in range(4):
                                    nc.tensor.matmul(
                                        ps[:],
                                        wyh[:, nt, ds, :],
                                        AT[:, ds, c * 512:(c + 1) * 512],
                                        start=(ds == 0),
                                        stop=(ds == 3),
                                    )
                                nc.scalar.activation(
                                    YT[:, nt, c * 512:(c + 1) * 512], ps[:], Act.Relu
                                )
                        nc.vector.tensor_tensor(YT[:], YT[:], XT[:], op=Alu.mult)

                        # -- vd partial: (y^T)^T @ enc, scaled by rsq_h at evac --
                        enchs = []
                        for ns in range(8):
                            e = encp.tile([P, D], bf16, tag="ench", name=f"ench_{layer}_{h}_{ns}")
                            nc.sync.dma_start(e[:], ENC[:, h, ns])
                            enchs.append(e)
                        for tt in range(8):
                            ps = mmps.tile([P, 512], f32, tag="mm", name="vd_ps")
                            for ns in range(8):
                                nc.tensor.matmul(
                                    ps[:],
                                    YT[:, ns, tt * 128:(tt + 1) * 128],
                                    enchs[ns][:],
                                    start=(ns == 0),
                                    stop=(ns == 7),
                                )
                            if h == 0:
                                nc.scalar.activation(
                                    vd[:, tt, :], ps[:], Act.Copy, scale=rsqh[:, tt:tt + 1]
                                )
                            else:
                                nc.vector.scalar_tensor_tensor(
                                    vd[:, tt, :], ps[:], rsqh[:, tt:tt + 1],
                                    vd[:, tt, :], Alu.mult, Alu.add,
                                )

                    # -- pair AllReduce of vd --
                    cc_in = drp.tile([T, D], f32, tag="cc_in", name=f"cc_in_{layer}")
                    cc_out = drp.tile([T, D], f32, tag="cc_out", name=f"cc_out_{layer}")
                    nc.sync.dma_start(cc_in.rearrange("(o p) d -> p o d", p=P), vd[:])
                    nc.gpsimd.collective_compute(
                        "AllReduce", Alu.add,
                        replica_groups=GROUPS,
                        ins=[cc_in[:]],
                        outs=[cc_out[:]],
                    )
                    nc.sync.dma_start(vd[:], cc_out.rearrange("(o p) d -> p o d", p=P))

                    # -- v = ln(v + ln(vd)) --
                    for tt in range(8):
                        s = spool.tile([P, 1], f32, tag="s")
                        nc.vector.tensor_reduce(s[:], vd[:, tt, :], axis=AX.X, op=Alu.add)
                        q = spool.tile([P, 1], f32, tag="q")
                        dummy = scrp.tile([P, D], bf16, tag="lndum", name="lndum")
                        nc.scalar.activation(dummy[:], vd[:, tt, :], Act.Square, accum_out=q[:])
                        mu = spool.tile([P, 1], f32, tag="mu")
                        nc.scalar.mul(mu[:], s[:], 1.0 / D)
                        mu2 = spool.tile([P, 1], f32, tag="mu2")
                        nc.vector.tensor_tensor(mu2[:], mu[:], mu[:], op=Alu.mult)
                        var = spool.tile([P, 1], f32, tag="var")
                        nc.vector.scalar_tensor_tensor(
                            var[:], q[:], 1.0 / D, mu2[:], Alu.mult, Alu.subtract
                        )
                        std = spool.tile([P, 1], f32, tag="std")
                        nc.scalar.activation(std[:], var[:], Act.Sqrt, bias=epsb[:])
                        rsq = spool.tile([P, 1], f32, tag="rsq")
                        nc.vector.reciprocal(rsq[:], std[:])
                        lnvd = scrp.tile([P, D], f32, tag="lnvd", name="lnvd")
                        nc.vector.tensor_scalar(
                            lnvd[:], vd[:, tt, :], mu[:], rsq[:], Alu.subtract, Alu.mult
                        )
                        # w = v + ln(vd); w rows are exactly zero-mean -> RMS only
                        nc.vector.tensor_tensor(v32[:, tt, :], v32[:, tt, :], lnvd[:], op=Alu.add)
                        q2 = spool.tile([P, 1], f32, tag="q2")
                        dummy2 = scrp.tile([P, D], bf16, tag="lndum", name="lndum2")
                        nc.scalar.activation(dummy2[:], v32[:, tt, :], Act.Square, accum_out=q2[:])
                        std2 = spool.tile([P, 1], f32, tag="std2")
                        nc.scalar.activation(std2[:], q2[:], Act.Sqrt, bias=epsb[:], scale=1.0 / D)
                        rsq2 = spool.tile([P, 1], f32, tag="rsq2")
                        nc.vector.reciprocal(rsq2[:], std2[:])
                        nc.vector.tensor_scalar(
                            v32[:, tt, :], v32[:, tt, :], rsq2[:], None, Alu.mult
                        )
                        nc.scalar.copy(vbf[:, tt, :], v32[:, tt, :])
                    transpose_v()

            # ---- lm head: logits = v @ lm_head_half ----
            with (
                tc.tile_pool(name="lmhp", bufs=2) as lmhp,
                tc.tile_pool(name="obp", bufs=3) as obp,
            ):
                CG = 2000  # vocab columns per output DMA (4 matmul chunks of 500)
                for cg in range(VLOC // CG):
                    rhs = lmhp.tile([P, 4, CG], bf16, tag="lmh", name=f"lmh_{cg}")
                    nc.sync.dma_start(rhs[:], LMH[:, :, cg * CG:(cg + 1) * CG])
                    for tt in range(8):
                        ob = obp.tile([P, CG], f32, tag="ob", name=f"ob_{cg}_{tt}")
                        for cc in range(CG // 500):
                            ps = mmps.tile([P, 512], f32, tag="mm", name="lm_ps")
                            for ds in range(4):
                                nc.tensor.matmul(
                                    ps[:, :500],
                                    vT[:, ds, tt * 128:(tt + 1) * 128],
                                    rhs[:, ds, cc * 500:(cc + 1) * 500],
                                    start=(ds == 0),
                                    stop=(ds == 3),
                                )
                            nc.scalar.copy(ob[:, cc * 500:(cc + 1) * 500], ps[:, :500])
                        nc.sync.dma_start(
                            OUT[tt * 128:(tt + 1) * 128, cg * CG:(cg + 1) * CG], ob[:]
                        )

    nc.finalize()
    return nc


# host-side input prep -------------------------------------------------------

_PERM = np.concatenate([np.arange(0, N, 2), np.arange(1, N, 2)])  # evens then odds


def prep_in_maps(idx, embed, decoder_x, decoder_y, encoder, lm_head):
    idx = np.asarray(idx).astype(np.int32)
    embed = np.ascontiguousarray(np.asarray(embed, dtype=np.float32))
    decoder_x = np.asarray(decoder_x, dtype=np.float32)
    decoder_y = np.asarray(decoder_y, dtype=np.float32)
    encoder = np.asarray(encoder, dtype=np.float32)
    lm_head = np.asarray(lm_head, dtype=np.float32)

    bf = ml_dtypes.bfloat16

    # rope tables, [pair-idx striped over (p, s), t]
    i = np.arange(N // 2, dtype=np.float64)
    freqs = 1.0 / (10000.0 ** (2.0 * i / N))          # (512,)
    ang = np.arange(T, dtype=np.float64)[None, :] * freqs[:, None]  # (512, T)
    cos_t = np.cos(ang).reshape(4, P, T).transpose(1, 0, 2).astype(bf)
    sin_t = np.sin(ang).reshape(4, P, T).transpose(1, 0, 2).astype(bf)

    # strict-causal diag masks: mask[p, k, j] = 1 if (128k + p) < j else 0
    pp = np.arange(P)[:, None, None]
    kk = np.arange(4)[None, :, None]
    jj = np.arange(512)[None, None, :]
    mask = ((128 * kk + pp) < jj).astype(np.float32)

    in_maps = []
    for c in range(N_CORES):
        b, tp = c // 2, c % 2
        hs = slice(tp * NHL, tp * NHL + NHL)

        idx_b = idx[b]                                     # (T,)
        idx_t = idx_b.reshape(T // P, P).T.copy()          # [P, 8]

        wx = decoder_x[hs][:, :, _PERM]                    # (4, D, N)
        wy = decoder_y[hs][:, :, _PERM]
        # -> [P, h, nt, ds, j]: w[h, ds*128+p, nt*128+j]
        def wlay(w):
            w = w.reshape(NHL, 4, P, 8, P)                 # (h, ds, p, nt, j)
            return np.ascontiguousarray(w.transpose(2, 0, 3, 1, 4)).astype(bf)

        enc = encoder.reshape(NH, N, D)[hs][:, _PERM, :]   # (4, N, D)
        enc = enc.reshape(NHL, 8, P, D)                    # (h, ns, p, d)
        enc_t = np.ascontiguousarray(enc.transpose(2, 0, 1, 3)).astype(bf)

        lmh = lm_head[:, tp * VLOC:(tp + 1) * VLOC]        # (D, VLOC)
        lmh = lmh.reshape(4, P, VLOC)                      # (ds, p, u)
        lmh_t = np.ascontiguousarray(lmh.transpose(1, 0, 2)).astype(bf)

        in_maps.append({
            "emb": embed,
            "idx": np.ascontiguousarray(idx_t),
            "wx": wlay(wx),
            "wy": wlay(wy),
            "enc": enc_t,
            "lmh": lmh_t,
            "cos": np.ascontiguousarray(cos_t),
            "sin": np.ascontiguousarray(sin_t),
            "mask": np.ascontiguousarray(mask),
        })
    return in_maps


_NC_CACHE = {}


def get_nc():
    if "nc" not in _NC_CACHE:
        _NC_CACHE["nc"] = build_nc()
    return _NC_CACHE["nc"]


def kernel(idx, embed, decoder_x, decoder_y, encoder, lm_head):
    idx = np.asarray(idx)
    in_maps = prep_in_maps(idx, embed, decoder_x, decoder_y, encoder, lm_head)
    nc = get_nc()
    res = run_bass_kernel_spmd(nc, in_maps, core_ids=list(range(N_CORES)))
    logits = np.empty((B, T, VOCAB), dtype=np.float32)
    for c in range(N_CORES):
        b, tp = c // 2, c % 2
        logits[b, :, tp * VLOC:(tp + 1) * VLOC] = res.results[c]["out"]
    return logits
